# revision 3
# baseline (speedup 1.0000x reference)
"""Trainium2 Bass kernel for the CaptionDecoder problem.

Math notes (derived from the reference):
  * The 4 stacked "attention" Linear layers have no nonlinearity, so
    scores = [enc, broadcast(h)] @ M + c with M = fc1_w.T fc2_w.T fc3_w.T fc4_w.T attn_w.
    The h-dependent part of scores is constant along S, and softmax is
    shift-invariant per row => attention weights (and thus ctx) are
    INDEPENDENT of h, i.e. constant across all 27 timesteps.
  * With training_steps=-200 teacher forcing always uses target embeddings.
  * Everything downstream is: one embedding-projection GEMM (batched over t),
    a 27-step LSTM recurrence (h @ w_hh.T per step), and one big output GEMM
    (27*B x 512) @ (512 x 8000).

Sharding: pure data-parallel over batch, 16 rows per core, no collectives.
"""

import numpy as np

import concourse.bass as bass
import concourse.tile as tile
from concourse import bacc, mybir
from concourse.bass import ds, ts
from concourse.bass_utils import run_bass_kernel_spmd
from concourse.masks import make_identity

F32 = mybir.dt.float32
I32 = mybir.dt.int32
AF = mybir.ActivationFunctionType
ALU = mybir.AluOpType
AX = mybir.AxisListType

# Global problem shapes (full batch)
B_FULL = 128
N_CORES = 8
B = B_FULL // N_CORES      # 16 local batch
S = 80
H = 512
WD = 1024
T = 27                     # T-1 steps of the original T=28
V = 8000
G = 4 * H                  # 2048 gates
R = T * B                  # 432 output rows per core, r = t*16+b for X, j = b*27+t for logits
KA = 640                   # 512 + bias row, padded to 5*128
NV = 500                   # vocab chunk for logits GEMM
NVT = V // NV              # 16

_CACHE = {}


def _build_module():
    nc = bacc.Bacc(None, target_bir_lowering=False, debug=False)

    # ---------------- DRAM I/O ----------------
    enc = nc.dram_tensor("enc", [B, S, H], F32, kind="ExternalInput")
    emb = nc.dram_tensor("emb", [V, WD], F32, kind="ExternalInput")
    idx = nc.dram_tensor("idx", [R, 1], I32, kind="ExternalInput")
    h0T = nc.dram_tensor("h0T", [H, B], F32, kind="ExternalInput")
    vrep = nc.dram_tensor("vrep", [128, H], F32, kind="ExternalInput")
    wihxT = nc.dram_tensor("wihxT", [WD, G], F32, kind="ExternalInput")
    wctxT = nc.dram_tensor("wctxT", [KA, G], F32, kind="ExternalInput")
    whhT = nc.dram_tensor("whhT", [H, G], F32, kind="ExternalInput")
    owT = nc.dram_tensor("owT", [KA, V], F32, kind="ExternalInput")
    e16 = nc.dram_tensor("e16", [128, 128], F32, kind="ExternalInput")
    logits = nc.dram_tensor("logits", [R, V], F32, kind="ExternalOutput")
    xb_dram = nc.dram_tensor("xb_scratch", [R, G], F32)

    KH = H // 128            # 4 k-tiles over H
    KW = WD // 128           # 8 k-tiles over WD
    KAT = KA // 128          # 5 k-tiles over augmented K
    MT = [128, 128, 128, 48]  # M-tile sizes over R

    with tile.TileContext(nc) as tc:
        with tc.tile_pool(name="persist", bufs=1) as pp:
            I128 = pp.tile([128, 128], F32, tag="I128")
            make_identity(nc, I128[:])

            HsT = pp.tile([128, KAT, R], F32, tag="HsT")
            bias_sb = pp.tile([128, G], F32, tag="bias_sb")
            whh_sb = pp.tile([128, KH, G], F32, tag="whh_sb")
            h0T_sb = pp.tile([128, KH, B], F32, tag="h0T_sb")
            e16_sb = pp.tile([128, 128], F32, tag="e16_sb")
            wtsT_sb = pp.tile([128, B], F32, tag="wtsT_sb")
            ctxT_sb = pp.tile([128, KH, B], F32, tag="ctxT_sb")
            c0_sb = pp.tile([B, H], F32, tag="c0_sb")

            nc.vector.memset(c0_sb[:], 0.0)
            # ones row (partition 0 of k-tile 4) for the bias rows of wctxT/owT
            nc.vector.memset(HsT[:, KAT - 1, :], 0.0)
            nc.vector.memset(HsT[:1, KAT - 1, :], 1.0)
            nc.sync.dma_start(e16_sb[:], e16[:])
            nc.sync.dma_start(h0T_sb[:], h0T[:].rearrange("(kk p) b -> p kk b", p=128))
            for kk in range(KH):
                nc.sync.dma_start(whh_sb[:, kk, :], whhT[ts(kk, 128), :])

            # ================= Phase A: attention (collapsed) =================
            with (
                tc.tile_pool(name="phA", bufs=1) as pa,
                tc.tile_pool(name="phA_st", bufs=2) as pas,
                tc.tile_pool(name="phA_ps", bufs=2, space="PSUM") as paps,
            ):
                vrep_sb = pa.tile([128, H], F32, tag="vrep_sb")
                nc.sync.dma_start(vrep_sb[:], vrep[:])
                enc_sb = pa.tile([128, B, H], F32, tag="enc_sb")
                nc.vector.memset(enc_sb[:], 0.0)
                for b in range(B):
                    nc.sync.dma_start(enc_sb[:S, b, :], enc[b, :, :])

                # escT[s, b] = sum_d enc[b,s,d] * v[d]
                escT = pa.tile([128, B], F32, tag="escT")
                nc.vector.memset(escT[:], 0.0)
                for b in range(B):
                    prod = pas.tile([S, H], F32, tag="prod")
                    nc.vector.tensor_tensor(prod[:], enc_sb[:S, b, :], vrep_sb[:S, :], op=ALU.mult)
                    nc.vector.tensor_reduce(escT[:S, b : b + 1], prod[:], axis=AX.X, op=ALU.add)

                # esc = escT.T -> [16, 128(S used)]
                esc_ps = paps.tile([B, 128], F32, tag="esc_ps")
                nc.tensor.transpose(esc_ps[:], escT[:], I128[:])

                # softmax over s (free dim), rows are b
                negmax = pa.tile([B, 1], F32, tag="negmax")
                sums = pa.tile([B, 1], F32, tag="sums")
                rec = pa.tile([B, 1], F32, tag="rec")
                wts = pa.tile([128, S], F32, tag="wts")
                nc.vector.memset(wts[:], 0.0)
                nc.vector.tensor_reduce(negmax[:], esc_ps[:, :S], axis=AX.X, op=ALU.max, negate=True)
                nc.scalar.activation(wts[:B, :], esc_ps[:, :S], AF.Exp, bias=negmax[:], scale=1.0,
                                     accum_out=sums[:])
                nc.vector.reciprocal(rec[:], sums[:])
                nc.vector.tensor_scalar_mul(wts[:B, :], wts[:B, :], rec[:])

                # wtsT [128(S used), 16] via PE transpose (K=128, wts zero-padded)
                wtsT_ps = paps.tile([S, 128], F32, tag="wtsT_ps")
                nc.vector.memset(wtsT_sb[:], 0.0)
                nc.tensor.transpose(wtsT_ps[:], wts[:], I128[:])
                nc.vector.tensor_copy(wtsT_sb[:S, :], wtsT_ps[:, :B])

                # ctxT[d, b] = sum_s wts[b,s] * enc[b,s,d]  (64 small matmuls)
                for kk in range(KH):
                    ctx_ps = paps.tile([128, B], F32, tag="ctx_ps")
                    for b in range(B):
                        nc.tensor.matmul(
                            ctx_ps[:, b : b + 1],
                            lhsT=enc_sb[:, b, ts(kk, 128)],
                            rhs=wtsT_sb[:, b : b + 1],
                            start=True, stop=True,
                        )
                    nc.vector.tensor_copy(ctxT_sb[:, kk, :], ctx_ps[:])

                # bias_eff = ctx @ w_ih[:,WD:].T + (b_ih + b_hh)   [16, 2048]
                nc.vector.memset(bias_sb[:], 0.0)
                wctx_tiles = []
                for kk in range(KAT):
                    wct = pa.tile([128, G], F32, tag=f"wct{kk}")
                    nc.sync.dma_start(wct[:], wctxT[ts(kk, 128), :])
                    wctx_tiles.append(wct)
                for n in range(G // 512):
                    pb = paps.tile([B, 512], F32, tag="pb")
                    for kk in range(KAT):
                        lhsT = ctxT_sb[:, kk, :] if kk < KH else HsT[:, KAT - 1, :B]
                        nc.tensor.matmul(pb[:], lhsT=lhsT, rhs=wctx_tiles[kk][:, ts(n, 512)],
                                         start=(kk == 0), stop=(kk == KAT - 1))
                    nc.vector.tensor_copy(bias_sb[:B, ts(n, 512)], pb[:])

            # ================= Phase B: embedding gather + X GEMM =============
            with (
                tc.tile_pool(name="phB", bufs=1) as pb_,
                tc.tile_pool(name="phB_gx", bufs=2) as pgx,
                tc.tile_pool(name="phB_ps", bufs=2, space="PSUM") as pbps,
            ):
                xT = pb_.tile([128, 4, KW, 128], F32, tag="xT")
                for j in range(4):
                    nj = MT[j]
                    idx_sb = pgx.tile([128, 1], I32, tag="idx_sb")
                    nc.sync.dma_start(idx_sb[:nj, :], idx[ds(j * 128, nj), :])
                    gx = pgx.tile([128, WD], F32, tag="gx")
                    if nj < 128:
                        nc.vector.memset(gx[:], 0.0)
                    nc.gpsimd.indirect_dma_start(
                        out=gx[:nj, :],
                        out_offset=None,
                        in_=emb[:],
                        in_offset=bass.IndirectOffsetOnAxis(ap=idx_sb[:nj, :1], axis=0),
                    )
                    for kk in range(KW):
                        tp = pbps.tile([128, 128], F32, tag="tp")
                        nc.tensor.transpose(tp[:], gx[:, ts(kk, 128)], I128[:])
                        nc.vector.tensor_copy(xT[:, j, kk, :nj], tp[:, :nj])

                wx_tiles = []
                for kk in range(KW):
                    wx = pb_.tile([128, G], F32, tag=f"wx{kk}")
                    nc.sync.dma_start(wx[:], wihxT[ts(kk, 128), :])
                    wx_tiles.append(wx)

                for j in range(4):
                    nj = MT[j]
                    for n in range(G // 512):
                        px = pbps.tile([128, 512], F32, tag="px")
                        for kk in range(KW):
                            nc.tensor.matmul(px[:nj, :], lhsT=xT[:, j, kk, :nj],
                                             rhs=wx_tiles[kk][:, ts(n, 512)],
                                             start=(kk == 0), stop=False)
                        # += broadcast-over-t of bias_eff (E16 trick), rows r=t*16+b
                        nc.tensor.matmul(px[:nj, :], lhsT=e16_sb[:, :nj],
                                         rhs=bias_sb[:, ts(n, 512)],
                                         start=False, stop=True)
                        xsb = pgx.tile([128, 512], F32, tag="xsb")
                        nc.vector.tensor_copy(xsb[:nj, :], px[:nj, :])
                        nc.sync.dma_start(xb_dram[ds(j * 128, nj), ts(n, 512)], xsb[:nj, :])

            # ================= Phase C: LSTM recurrence ======================
            HsT_r = HsT[:].rearrange("p k (b t) -> p k b t", t=T)
            with (
                tc.tile_pool(name="phC", bufs=3) as pc,
                tc.tile_pool(name="phC_g", bufs=3, space="PSUM") as pcg,
                tc.tile_pool(name="phC_t", bufs=2, space="PSUM") as pct,
            ):
                c_prev = c0_sb
                for t in range(T):
                    xsl = pc.tile([B, G], F32, tag="xsl")
                    nc.sync.dma_start(xsl[:], xb_dram[ds(t * B, B), :])
                    g_sb = []
                    for n in range(4):
                        pg = pcg.tile([B, 512], F32, tag="pg")
                        for kk in range(KH):
                            lhsT = h0T_sb[:, kk, :] if t == 0 else HsT_r[:, kk, :, t - 1]
                            nc.tensor.matmul(pg[:], lhsT=lhsT, rhs=whh_sb[:, kk, ts(n, 512)],
                                             start=(kk == 0), stop=(kk == KH - 1))
                        gs = pc.tile([B, 512], F32, tag=f"gs{n}")
                        nc.vector.tensor_tensor(gs[:], pg[:], xsl[:, ts(n, 512)], op=ALU.add)
                        g_sb.append(gs)

                    a_i = pc.tile([B, 512], F32, tag="a_i")
                    a_f = pc.tile([B, 512], F32, tag="a_f")
                    a_g = pc.tile([B, 512], F32, tag="a_g")
                    a_o = pc.tile([B, 512], F32, tag="a_o")
                    nc.scalar.activation(a_i[:], g_sb[0][:], AF.Sigmoid)
                    nc.scalar.activation(a_f[:], g_sb[1][:], AF.Sigmoid)
                    nc.scalar.activation(a_g[:], g_sb[2][:], AF.Tanh)
                    nc.scalar.activation(a_o[:], g_sb[3][:], AF.Sigmoid)

                    fc_ = pc.tile([B, 512], F32, tag="fc_")
                    ig_ = pc.tile([B, 512], F32, tag="ig_")
                    c_new = pc.tile([B, 512], F32, tag="c_new")
                    tc_ = pc.tile([B, 512], F32, tag="tc_")
                    nc.vector.tensor_tensor(fc_[:], a_f[:], c_prev[:], op=ALU.mult)
                    nc.vector.tensor_tensor(ig_[:], a_i[:], a_g[:], op=ALU.mult)
                    nc.vector.tensor_tensor(c_new[:], fc_[:], ig_[:], op=ALU.add)
                    nc.scalar.activation(tc_[:], c_new[:], AF.Tanh)

                    h_pad = pc.tile([128, H], F32, tag="h_pad")
                    nc.vector.memset(h_pad[:], 0.0)
                    nc.vector.tensor_tensor(h_pad[:B, :], a_o[:], tc_[:], op=ALU.mult)

                    for kk in range(KH):
                        tp = pct.tile([128, 128], F32, tag="tph")
                        nc.tensor.transpose(tp[:], h_pad[:, ts(kk, 128)], I128[:])
                        nc.vector.tensor_copy(HsT_r[:, kk, :, t], tp[:, :B])
                    c_prev = c_new

            # ================= Phase D: logits GEMM ==========================
            with (
                tc.tile_pool(name="phD_w", bufs=3) as pdw,
                tc.tile_pool(name="phD_o", bufs=3) as pdo,
                tc.tile_pool(name="phD_ps", bufs=2, space="PSUM") as pdps,
            ):
                for n in range(NVT):
                    ow = pdw.tile([128, KAT, NV], F32, tag="ow")
                    for kk in range(KAT):
                        nc.sync.dma_start(ow[:, kk, :], owT[ts(kk, 128), ds(n * NV, NV)])
                    for m in range(4):
                        nm = MT[m]
                        pl = pdps.tile([128, NV], F32, tag="pl")
                        for kk in range(KAT):
                            nc.tensor.matmul(pl[:nm, :], lhsT=HsT[:, kk, ds(m * 128, nm)],
                                             rhs=ow[:, kk, :],
                                             start=(kk == 0), stop=(kk == KAT - 1))
                        lo = pdo.tile([128, NV], F32, tag="lo")
                        nc.vector.tensor_copy(lo[:nm, :], pl[:nm, :])
                        nc.sync.dma_start(logits[ds(m * 128, nm), ds(n * NV, NV)], lo[:nm, :])

    nc.compile()
    return nc


def _prep_host(inputs):
    """Host-side input prep: weight transposes/folds + per-core shards."""
    f = lambda k: np.asarray(inputs[k], np.float32)
    enc_full = f("encoder_outputs")                      # (128, 80, 512)
    h0_full = f("encoder_hidden_state")[0]               # (128, 512)
    tgt = np.asarray(inputs["target_sentences"]).astype(np.int64)  # (128, 28)
    embw = f("emb")
    w_ih, w_hh = f("w_ih"), f("w_hh")
    b_ih, b_hh = f("b_ih"), f("b_hh")
    out_w, out_b = f("out_w"), f("out_b")

    # collapsed attention read-out vector  v = fc1^T fc2^T fc3^T fc4^T a
    v = np.asarray(inputs["attn_w"], np.float64)
    for wname in ("fc4_w", "fc3_w", "fc2_w", "fc1_w"):
        v = np.asarray(inputs[wname], np.float64).T @ v
    v_enc = v[:H].astype(np.float32)                     # (512,)
    vrep = np.ascontiguousarray(np.broadcast_to(v_enc, (128, H)))

    wihxT = np.ascontiguousarray(w_ih[:, :WD].T)         # (1024, 2048)
    wctxT = np.zeros((KA, G), np.float32)
    wctxT[:H] = w_ih[:, WD:].T
    wctxT[H] = b_ih + b_hh
    whhT = np.ascontiguousarray(w_hh.T)                  # (512, 2048)
    owT = np.zeros((KA, V), np.float32)
    owT[:H] = out_w.T
    owT[H] = out_b
    e16 = np.zeros((128, 128), np.float32)
    for r in range(128):
        e16[r % B, r] = 1.0

    shared = {"emb": embw, "vrep": vrep, "wihxT": wihxT, "wctxT": wctxT,
              "whhT": whhT, "owT": owT, "e16": e16}
    in_maps = []
    for c in range(N_CORES):
        bs = slice(c * B, (c + 1) * B)
        tloc = tgt[bs, :T]                               # (16, 27), steps 0..26
        # gather order r = t*16 + b  (t-major, matches X GEMM M-tiles)
        idx = np.ascontiguousarray(tloc.T.reshape(R, 1)).astype(np.int32)
        m = dict(shared)
        m["enc"] = np.ascontiguousarray(enc_full[bs])
        m["h0T"] = np.ascontiguousarray(h0_full[bs].T)
        m["idx"] = idx
        in_maps.append(m)
    return in_maps


def kernel(**inputs):
    if "nc" not in _CACHE:
        _CACHE["nc"] = _build_module()
    nc = _CACHE["nc"]
    in_maps = _prep_host(inputs)
    res = run_bass_kernel_spmd(nc, in_maps, core_ids=list(range(N_CORES)))
    # rows j = b*27 + t  ->  (16, 27, 8000) per core
    parts = [res.results[c]["logits"].reshape(B, T, V) for c in range(N_CORES)]
    logits = np.concatenate(parts, axis=0)               # (128, 27, 8000)
    preds = np.argmax(logits, axis=2).astype(np.int32)
    return logits, preds


# revision 7
# speedup vs baseline: 1.7572x; 1.7572x over previous
"""Trainium2 Bass kernel for the CaptionDecoder problem.

Math notes (derived from the reference):
  * The 4 stacked "attention" Linear layers have no nonlinearity, so
    scores = [enc, broadcast(h)] @ M + c with M = fc1_w.T fc2_w.T fc3_w.T fc4_w.T attn_w.
    The h-dependent part of scores is constant along S, and softmax is
    shift-invariant per row => attention weights (and thus ctx) are
    INDEPENDENT of h, i.e. constant across all 27 timesteps.
  * With training_steps=-200 teacher forcing always uses target embeddings.
  * Everything downstream is: one embedding-projection GEMM (batched over t),
    a 27-step LSTM recurrence (h @ w_hh.T per step), and one big output GEMM
    (27*B x 512) @ (512 x 8000).

Sharding: pure data-parallel over batch, 16 rows per core, no collectives.
"""

import numpy as np

import concourse.bass as bass
import concourse.tile as tile
from concourse import bacc, mybir
from concourse.bass import ds, ts
from concourse.bass_utils import run_bass_kernel_spmd
from concourse.masks import make_identity

F32 = mybir.dt.float32
F32R = mybir.dt.float32r
I32 = mybir.dt.int32
AF = mybir.ActivationFunctionType
ALU = mybir.AluOpType
AX = mybir.AxisListType

# Global problem shapes (full batch)
B_FULL = 128
N_CORES = 8
B = B_FULL // N_CORES      # 16 local batch
S = 80
H = 512
WD = 1024
T = 27                     # T-1 steps of the original T=28
V = 8000
G = 4 * H                  # 2048 gates
R = T * B                  # 432 output rows per core, r = t*16+b for X, j = b*27+t for logits
KA = 640                   # 512 + bias row, padded to 5*128
NV = 500                   # vocab chunk for logits GEMM
NVT = V // NV              # 16

_CACHE = {}


def _build_module():
    nc = bacc.Bacc(None, target_bir_lowering=False, debug=False)

    # ---------------- DRAM I/O ----------------
    enc = nc.dram_tensor("enc", [B, S, H], F32, kind="ExternalInput")
    emb = nc.dram_tensor("emb", [V, WD], F32, kind="ExternalInput")
    idx = nc.dram_tensor("idx", [R, 1], I32, kind="ExternalInput")
    h0T = nc.dram_tensor("h0T", [H, B], F32R, kind="ExternalInput")
    vrep = nc.dram_tensor("vrep", [128, H], F32, kind="ExternalInput")
    wihxT = nc.dram_tensor("wihxT", [WD, G], F32R, kind="ExternalInput")
    wctxT = nc.dram_tensor("wctxT", [KA, G], F32R, kind="ExternalInput")
    whhT = nc.dram_tensor("whhT", [H, G], F32R, kind="ExternalInput")
    owT = nc.dram_tensor("owT", [KA, V], F32R, kind="ExternalInput")
    e16 = nc.dram_tensor("e16", [128, 128], F32R, kind="ExternalInput")
    logits = nc.dram_tensor("logits", [R, V], F32, kind="ExternalOutput")
    xb_dram = nc.dram_tensor("xb_scratch", [R, G], F32)

    KH = H // 128            # 4 k-tiles over H
    KW = WD // 128           # 8 k-tiles over WD
    KAT = KA // 128          # 5 k-tiles over augmented K
    MT = [128, 128, 128, 48]  # M-tile sizes over R

    with tile.TileContext(nc) as tc:
        with tc.tile_pool(name="persist", bufs=1) as pp:
            I128 = pp.tile([128, 128], F32, tag="I128")
            make_identity(nc, I128[:])

            HsT = pp.tile([128, KAT, R], F32R, tag="HsT")
            bias_sb = pp.tile([128, G], F32R, tag="bias_sb")
            whh_sb = pp.tile([128, KH, G], F32R, tag="whh_sb")
            h0T_sb = pp.tile([128, KH, B], F32R, tag="h0T_sb")
            e16_sb = pp.tile([128, 128], F32R, tag="e16_sb")
            wtsT_sb = pp.tile([128, B], F32, tag="wtsT_sb")
            ctxT_sb = pp.tile([128, KH, B], F32R, tag="ctxT_sb")
            c0_sb = pp.tile([B, H], F32, tag="c0_sb")

            nc.vector.memset(c0_sb[:], 0.0)
            # ones row (partition 0 of k-tile 4) for the bias rows of wctxT/owT
            nc.vector.memset(HsT[:, KAT - 1, :].bitcast(F32), 0.0)
            nc.vector.memset(HsT[:1, KAT - 1, :].bitcast(F32), 1.0)
            nc.sync.dma_start(e16_sb[:], e16[:])
            nc.sync.dma_start(h0T_sb[:], h0T[:].rearrange("(kk p) b -> p kk b", p=128))
            for kk in range(KH):
                nc.sync.dma_start(whh_sb[:, kk, :], whhT[ts(kk, 128), :])

            # ================= Phase A: attention (collapsed) =================
            with (
                tc.tile_pool(name="phA", bufs=1) as pa,
                tc.tile_pool(name="phA_st", bufs=2) as pas,
                tc.tile_pool(name="phA_ps", bufs=2, space="PSUM") as paps,
            ):
                vrep_sb = pa.tile([128, H], F32, tag="vrep_sb")
                nc.sync.dma_start(vrep_sb[:], vrep[:])
                enc_sb = pa.tile([128, B, H], F32, tag="enc_sb")
                nc.vector.memset(enc_sb[:], 0.0)
                for b in range(B):
                    nc.sync.dma_start(enc_sb[:S, b, :], enc[b, :, :])

                # escT[s, b] = sum_d enc[b,s,d] * v[d]
                escT = pa.tile([128, B], F32, tag="escT")
                nc.vector.memset(escT[:], 0.0)
                for b in range(B):
                    prod = pas.tile([S, H], F32, tag="prod")
                    nc.vector.tensor_tensor(prod[:], enc_sb[:S, b, :], vrep_sb[:S, :], op=ALU.mult)
                    nc.vector.tensor_reduce(escT[:S, b : b + 1], prod[:], axis=AX.X, op=ALU.add)

                # esc = escT.T -> [16, 128(S used)]
                esc_ps = paps.tile([B, 128], F32, tag="esc_ps")
                nc.tensor.transpose(esc_ps[:], escT[:], I128[:])

                # softmax over s (free dim), rows are b
                negmax = pa.tile([B, 1], F32, tag="negmax")
                sums = pa.tile([B, 1], F32, tag="sums")
                rec = pa.tile([B, 1], F32, tag="rec")
                wts = pa.tile([128, S], F32, tag="wts")
                nc.vector.memset(wts[:], 0.0)
                nc.vector.tensor_reduce(negmax[:], esc_ps[:, :S], axis=AX.X, op=ALU.max, negate=True)
                nc.scalar.activation(wts[:B, :], esc_ps[:, :S], AF.Exp, bias=negmax[:], scale=1.0,
                                     accum_out=sums[:])
                nc.vector.reciprocal(rec[:], sums[:])
                nc.vector.tensor_scalar_mul(wts[:B, :], wts[:B, :], rec[:])

                # wtsT [128(S used), 16] via PE transpose (K=128, wts zero-padded)
                wtsT_ps = paps.tile([S, 128], F32, tag="wtsT_ps")
                nc.vector.memset(wtsT_sb[:], 0.0)
                nc.tensor.transpose(wtsT_ps[:], wts[:], I128[:])
                nc.vector.tensor_copy(wtsT_sb[:S, :], wtsT_ps[:, :B])

                # ctxT[d, b] = sum_s wts[b,s] * enc[b,s,d]  (64 small matmuls)
                for kk in range(KH):
                    ctx_ps = paps.tile([128, B], F32, tag="ctx_ps")
                    for b in range(B):
                        nc.tensor.matmul(
                            ctx_ps[:, b : b + 1],
                            lhsT=enc_sb[:, b, ts(kk, 128)],
                            rhs=wtsT_sb[:, b : b + 1],
                            start=True, stop=True,
                        )
                    nc.vector.tensor_copy(ctxT_sb[:, kk, :], ctx_ps[:])

                # bias_eff = ctx @ w_ih[:,WD:].T + (b_ih + b_hh)   [16, 2048]
                nc.vector.memset(bias_sb[:].bitcast(F32), 0.0)
                wctx_tiles = []
                for kk in range(KAT):
                    wct = pa.tile([128, G], F32R, tag=f"wct{kk}")
                    nc.sync.dma_start(wct[:], wctxT[ts(kk, 128), :])
                    wctx_tiles.append(wct)
                for n in range(G // 512):
                    pb = paps.tile([B, 512], F32, tag="pb")
                    for kk in range(KAT):
                        lhsT = ctxT_sb[:, kk, :] if kk < KH else HsT[:, KAT - 1, :B]
                        nc.tensor.matmul(pb[:], lhsT=lhsT,
                                         rhs=wctx_tiles[kk][:, ts(n, 512)],
                                         start=(kk == 0), stop=(kk == KAT - 1))
                    nc.vector.tensor_copy(bias_sb[:B, ts(n, 512)], pb[:])

            # ================= Phase B: embedding gather + X GEMM =============
            with (
                tc.tile_pool(name="phB", bufs=1) as pb_,
                tc.tile_pool(name="phB_gx", bufs=2) as pgx,
                tc.tile_pool(name="phB_ps", bufs=2, space="PSUM") as pbps,
            ):
                xT = pb_.tile([128, 4, KW, 128], F32R, tag="xT")
                for j in range(4):
                    nj = MT[j]
                    idx_sb = pgx.tile([128, 1], I32, tag="idx_sb")
                    nc.sync.dma_start(idx_sb[:nj, :], idx[ds(j * 128, nj), :])
                    gx = pgx.tile([128, WD], F32, tag="gx")
                    if nj < 128:
                        nc.vector.memset(gx[:], 0.0)
                    nc.gpsimd.indirect_dma_start(
                        out=gx[:nj, :],
                        out_offset=None,
                        in_=emb[:],
                        in_offset=bass.IndirectOffsetOnAxis(ap=idx_sb[:nj, :1], axis=0),
                    )
                    for kk in range(KW):
                        tp = pbps.tile([128, 128], F32, tag="tp")
                        nc.tensor.transpose(tp[:], gx[:, ts(kk, 128)], I128[:])
                        nc.vector.tensor_copy(xT[:, j, kk, :nj], tp[:, :nj])

                wx_tiles = []
                for kk in range(KW):
                    wx = pb_.tile([128, G], F32R, tag=f"wx{kk}")
                    nc.sync.dma_start(wx[:], wihxT[ts(kk, 128), :])
                    wx_tiles.append(wx)

                for j in range(4):
                    nj = MT[j]
                    for n in range(G // 512):
                        px = pbps.tile([128, 512], F32, tag="px")
                        for kk in range(KW):
                            nc.tensor.matmul(px[:nj, :], lhsT=xT[:, j, kk, :nj],
                                             rhs=wx_tiles[kk][:, ts(n, 512)],
                                             start=(kk == 0), stop=False)
                        # += broadcast-over-t of bias_eff (E16 trick), rows r=t*16+b
                        nc.tensor.matmul(px[:nj, :], lhsT=e16_sb[:, :nj],
                                         rhs=bias_sb[:, ts(n, 512)],
                                         start=False, stop=True)
                        xsb = pgx.tile([128, 512], F32, tag="xsb")
                        nc.vector.tensor_copy(xsb[:nj, :], px[:nj, :])
                        nc.sync.dma_start(xb_dram[ds(j * 128, nj), ts(n, 512)], xsb[:nj, :])

            # ================= Phase C: LSTM recurrence ======================
            HsT_r = HsT[:].rearrange("p k (b t) -> p k b t", t=T)
            with (
                tc.tile_pool(name="phC", bufs=3) as pc,
                tc.tile_pool(name="phC_g", bufs=3, space="PSUM") as pcg,
                tc.tile_pool(name="phC_t", bufs=2, space="PSUM") as pct,
            ):
                c_prev = c0_sb
                for t in range(T):
                    xsl = pc.tile([B, G], F32, tag="xsl")
                    nc.sync.dma_start(xsl[:], xb_dram[ds(t * B, B), :])
                    g_sb = []
                    for n in range(4):
                        pg = pcg.tile([B, 512], F32, tag="pg")
                        for kk in range(KH):
                            lhsT = h0T_sb[:, kk, :] if t == 0 else HsT_r[:, kk, :, t - 1]
                            nc.tensor.matmul(pg[:], lhsT=lhsT,
                                             rhs=whh_sb[:, kk, ts(n, 512)],
                                             start=(kk == 0), stop=(kk == KH - 1))
                        gs = pc.tile([B, 512], F32, tag=f"gs{n}")
                        nc.vector.tensor_tensor(gs[:], pg[:], xsl[:, ts(n, 512)], op=ALU.add)
                        g_sb.append(gs)

                    a_i = pc.tile([B, 512], F32, tag="a_i")
                    a_f = pc.tile([B, 512], F32, tag="a_f")
                    a_g = pc.tile([B, 512], F32, tag="a_g")
                    a_o = pc.tile([B, 512], F32, tag="a_o")
                    nc.scalar.activation(a_i[:], g_sb[0][:], AF.Sigmoid)
                    nc.scalar.activation(a_f[:], g_sb[1][:], AF.Sigmoid)
                    nc.scalar.activation(a_g[:], g_sb[2][:], AF.Tanh)
                    nc.scalar.activation(a_o[:], g_sb[3][:], AF.Sigmoid)

                    fc_ = pc.tile([B, 512], F32, tag="fc_")
                    ig_ = pc.tile([B, 512], F32, tag="ig_")
                    c_new = pc.tile([B, 512], F32, tag="c_new")
                    tc_ = pc.tile([B, 512], F32, tag="tc_")
                    nc.vector.tensor_tensor(fc_[:], a_f[:], c_prev[:], op=ALU.mult)
                    nc.vector.tensor_tensor(ig_[:], a_i[:], a_g[:], op=ALU.mult)
                    nc.vector.tensor_tensor(c_new[:], fc_[:], ig_[:], op=ALU.add)
                    nc.scalar.activation(tc_[:], c_new[:], AF.Tanh)

                    h_pad = pc.tile([128, H], F32, tag="h_pad")
                    nc.vector.memset(h_pad[:], 0.0)
                    nc.vector.tensor_tensor(h_pad[:B, :], a_o[:], tc_[:], op=ALU.mult)

                    for kk in range(KH):
                        tp = pct.tile([128, 128], F32, tag="tph")
                        nc.tensor.transpose(tp[:], h_pad[:, ts(kk, 128)], I128[:])
                        nc.vector.tensor_copy(HsT_r[:, kk, :, t], tp[:, :B])
                    c_prev = c_new

            # ================= Phase D: logits GEMM ==========================
            with (
                tc.tile_pool(name="phD_w", bufs=3) as pdw,
                tc.tile_pool(name="phD_o", bufs=3) as pdo,
                tc.tile_pool(name="phD_ps", bufs=2, space="PSUM") as pdps,
            ):
                for n in range(NVT):
                    ow = pdw.tile([128, KAT, NV], F32R, tag="ow")
                    for kk in range(KAT):
                        nc.sync.dma_start(ow[:, kk, :], owT[ts(kk, 128), ds(n * NV, NV)])
                    for m in range(4):
                        nm = MT[m]
                        pl = pdps.tile([128, NV], F32, tag="pl")
                        for kk in range(KAT):
                            nc.tensor.matmul(pl[:nm, :], lhsT=HsT[:, kk, ds(m * 128, nm)],
                                             rhs=ow[:, kk, :],
                                             start=(kk == 0), stop=(kk == KAT - 1))
                        lo = pdo.tile([128, NV], F32, tag="lo")
                        nc.vector.tensor_copy(lo[:nm, :], pl[:nm, :])
                        nc.sync.dma_start(logits[ds(m * 128, nm), ds(n * NV, NV)], lo[:nm, :])

    nc.compile()
    return nc


def _prep_host(inputs):
    """Host-side input prep: weight transposes/folds + per-core shards."""
    f = lambda k: np.asarray(inputs[k], np.float32)
    enc_full = f("encoder_outputs")                      # (128, 80, 512)
    h0_full = f("encoder_hidden_state")[0]               # (128, 512)
    tgt = np.asarray(inputs["target_sentences"]).astype(np.int64)  # (128, 28)
    embw = f("emb")
    w_ih, w_hh = f("w_ih"), f("w_hh")
    b_ih, b_hh = f("b_ih"), f("b_hh")
    out_w, out_b = f("out_w"), f("out_b")

    # collapsed attention read-out vector  v = fc1^T fc2^T fc3^T fc4^T a
    v = np.asarray(inputs["attn_w"], np.float64)
    for wname in ("fc4_w", "fc3_w", "fc2_w", "fc1_w"):
        v = np.asarray(inputs[wname], np.float64).T @ v
    v_enc = v[:H].astype(np.float32)                     # (512,)
    vrep = np.ascontiguousarray(np.broadcast_to(v_enc, (128, H)))

    wihxT = np.ascontiguousarray(w_ih[:, :WD].T)         # (1024, 2048)
    wctxT = np.zeros((KA, G), np.float32)
    wctxT[:H] = w_ih[:, WD:].T
    wctxT[H] = b_ih + b_hh
    whhT = np.ascontiguousarray(w_hh.T)                  # (512, 2048)
    owT = np.zeros((KA, V), np.float32)
    owT[:H] = out_w.T
    owT[H] = out_b
    e16 = np.zeros((128, 128), np.float32)
    for r in range(128):
        e16[r % B, r] = 1.0

    shared = {"emb": embw, "vrep": vrep, "wihxT": wihxT, "wctxT": wctxT,
              "whhT": whhT, "owT": owT, "e16": e16}
    in_maps = []
    for c in range(N_CORES):
        bs = slice(c * B, (c + 1) * B)
        tloc = tgt[bs, :T]                               # (16, 27), steps 0..26
        # gather order r = t*16 + b  (t-major, matches X GEMM M-tiles)
        idx = np.ascontiguousarray(tloc.T.reshape(R, 1)).astype(np.int32)
        m = dict(shared)
        m["enc"] = np.ascontiguousarray(enc_full[bs])
        m["h0T"] = np.ascontiguousarray(h0_full[bs].T)
        m["idx"] = idx
        in_maps.append(m)
    return in_maps


def kernel(**inputs):
    if "nc" not in _CACHE:
        _CACHE["nc"] = _build_module()
    nc = _CACHE["nc"]
    in_maps = _prep_host(inputs)
    res = run_bass_kernel_spmd(nc, in_maps, core_ids=list(range(N_CORES)))
    # rows j = b*27 + t  ->  (16, 27, 8000) per core
    parts = [res.results[c]["logits"].reshape(B, T, V) for c in range(N_CORES)]
    logits = np.concatenate(parts, axis=0)               # (128, 27, 8000)
    preds = np.argmax(logits, axis=2).astype(np.int32)
    return logits, preds
